# revision 2
# baseline (speedup 1.0000x reference)
"""CondConv3d kernel for 8 TRN2 NeuronCores.

Math: the reference einsum 'bi,eocdwh->bocdwh' shares no index between
routing_weights and weight, so it factorizes:
    eff_kernel[b] = (sum_i routing[b,i]) * (sum_e weight[e])
    eff_bias[b]   = (sum_i routing[b,i]) * (sum_e bias[e])
=> out[b] = conv3d(x[b], s_b * W_sum, pad=1) + s_b * bias_sum

Sharding: data-parallel over batch B=8, one sample per core. The
per-sample scalar s_b is folded into that core's weights/bias on host.

Per-core kernel (bf16 inputs, fp32 accumulate, bf16 output):
  - x is padded ON HOST to the exact SBUF slot layout
    [CI, D+2, 68*66]: depth slices 0 and 17 are zeros, each slice is a
    68x66 grid with content at rows 2..65, cols 0..63.  So SBUF needs
    no memset and every fill is one contiguous-per-partition DMA.
  - SBUF holds the full padded x as [96, D, SLOT]: partition p = 32*kd
    + ci, slot d (full residency, no ring).  fill(d) is a single DMA
    whose DRAM source reads depth window [d, d+3) for the 3 kd-blocks.
  - conv = 9 PSUM-accumulated matmuls per chunk (kh,kw taps via
    free-dim address shifts of +-66/+-1), contraction 96 = (kd, C_in).
  - 4 depth slices run concurrently via PE column tiling
    (tile_position=(0,32j)); loop order (tap, chunk, j) keeps the 4 PE
    column tiles interleaved while weights stay constant per tap.
  - drain: ScalarE/VectorE copy PSUM->SBUF bf16 with per-partition
    bias add, stripping the 2 pad columns.
  - output: bf16, 4 dma_starts per 4-slice group (one per depth slice)
    so stores spread over all 16 DMA engines; host upcasts to f32.
"""

import sys

if "/opt/trn_rl_repo" not in sys.path:
    sys.path.insert(0, "/opt/trn_rl_repo")

import numpy as np
import ml_dtypes

import concourse.bass as bass
import concourse.tile as tile
from concourse import bacc, mybir
from concourse.bass_utils import run_bass_kernel_spmd

# problem shape (hardcoded per contest rules)
B, CI, CO, D, H, W = 8, 32, 32, 16, 64, 64
K = 3
NCORES = 8

# padded slot layout
WP = 66                 # padded row width (64 valid + 2 zero cols)
SLOT_ROWS = 68          # 2 zero rows, 64 content rows, 2 zero rows
SLOT = SLOT_ROWS * WP   # 4488 elements per depth-slice per partition
Q0 = 2 * WP             # content base offset inside a slot
DP = D + 2              # depth-padded slice count in DRAM

NSTEP = 9               # (kh, kw) taps
ROWS_PER_CHUNK = 7
CHUNK = ROWS_PER_CHUNK * WP  # 462 <= 512 (one PSUM bank)
# chunk start rows; last chunk overlaps (recomputes rows 57..62, drains row 63)
CHUNK_R0 = [0, 7, 14, 21, 28, 35, 42, 49, 56, 57]
# PSUM passes: chunks resident together while the 9 taps accumulate
PASSES = [(0, 1, 2, 3), (4, 5, 6, 7), (8, 9)]

F32 = mybir.dt.float32
BF16 = mybir.dt.bfloat16

_CACHE = {}


def _build_nc():
    # Bacc (vs raw Bass) runs the wait-fixup passes: an ISA instruction can
    # carry only 1 semaphore wait; Bacc spills extras to ldweights/events.
    nc = bacc.Bacc(None)
    x_d = nc.declare_dram_parameter("x", [CI, DP, SLOT], BF16, isOutput=False)
    w_d = nc.declare_dram_parameter("w", [96, NSTEP * CO], BF16, isOutput=False)
    b_d = nc.declare_dram_parameter("bias", [128, 1], F32, isOutput=False)
    o_d = nc.declare_dram_parameter("out", [CO, D * H * W], BF16, isOutput=True)

    with tile.TileContext(nc) as tc:
        with (
            tc.tile_pool(name="const", bufs=1) as const,
            tc.tile_pool(name="outs", bufs=2) as outp,
            tc.tile_pool(name="psum", bufs=8, space="PSUM") as psump,
        ):
            xp = const.tile([96, D, SLOT], BF16)
            wsb = const.tile([96, NSTEP, CO], BF16)
            bsb = const.tile([128, 1], F32)

            nc.sync.dma_start(
                out=wsb[:, :, :],
                in_=w_d[:].rearrange("p (s o) -> p s o", s=NSTEP),
            )
            nc.sync.dma_start(out=bsb[:, :], in_=b_d[:])

            def fill(d):
                """One DMA: depth window [d, d+3) of host-padded x feeds
                the 3 kd partition blocks of slot d."""
                src = bass.AP(
                    tensor=x_d,
                    offset=d * SLOT,
                    ap=[[SLOT, 3], [DP * SLOT, CI], [1, SLOT]],
                )
                nc.sync.dma_start(out=xp[0:96, d, :], in_=src)

            drain_ctr = [0]

            def drain(c, ps, ob):
                r0 = CHUNK_R0[c]
                ps3 = ps[:, :].rearrange("p (h w) -> p h w", h=ROWS_PER_CHUNK)
                if r0 == 57:
                    src3 = ps3[:, 6:7, 0:64]
                    dst3 = ob[:, 63 * 64 : 64 * 64].rearrange(
                        "p (h w) -> p h w", h=1
                    )
                else:
                    src3 = ps3[:, :, 0:64]
                    dst3 = ob[:, r0 * 64 : (r0 + 7) * 64].rearrange(
                        "p (h w) -> p h w", h=ROWS_PER_CHUNK
                    )
                if drain_ctr[0] % 2 == 0:
                    nc.scalar.activation(
                        out=dst3,
                        in_=src3,
                        func=mybir.ActivationFunctionType.Identity,
                        bias=bsb[:, :],
                        scale=1.0,
                    )
                else:
                    nc.vector.tensor_scalar_add(dst3, src3, bsb[:, :])
                drain_ctr[0] += 1

            def compute_group(g):
                ob = outp.tile([128, H * W], BF16)
                for pass_chunks in PASSES:
                    tiles = {}
                    for c in pass_chunks:
                        # full 512-float bank so every tile is bank-aligned
                        ps_full = psump.tile([128, 512], F32)
                        tiles[c] = ps_full[:, 0:CHUNK]
                    for s in range(NSTEP):
                        kh, kw = s // 3, s % 3
                        off = (kh - 1) * WP + (kw - 1)
                        for c in pass_chunks:
                            base = Q0 + CHUNK_R0[c] * WP + off
                            for j in range(4):
                                d = 4 * g + j
                                rhs = xp[0:96, d, base : base + CHUNK]
                                nc.tensor.matmul(
                                    out=tiles[c][32 * j : 32 * j + 32, :],
                                    lhsT=wsb[0:96, s, :],
                                    rhs=rhs,
                                    start=(s == 0),
                                    stop=(s == NSTEP - 1),
                                    tile_position=(0, 32 * j),
                                    # sim's group tracker is bank-coarse; the
                                    # 4 col-tiles run disjoint partition ranges
                                    skip_group_check=True,
                                )
                    for c in pass_chunks:
                        drain(c, tiles[c], ob)
                # ob partitions are (j, o); one store per depth slice so the
                # 4 dma_starts land on different queues (16 DMA engines)
                for j in range(4):
                    d = 4 * g + j
                    dst = bass.AP(
                        tensor=o_d,
                        offset=d * (H * W),
                        ap=[[D * H * W, CO], [1, H * W]],
                    )
                    nc.sync.dma_start(out=dst, in_=ob[32 * j : 32 * j + 32, :])

            for d in range(4):
                fill(d)
            for g in range(4):
                for d in range(4 * g + 4, min(4 * g + 8, D)):
                    fill(d)
                compute_group(g)

    nc.finalize()  # Bacc: runs wait-spill + register allocation passes
    return nc


def _get_nc():
    if "nc" not in _CACHE:
        _CACHE["nc"] = _build_nc()
    return _CACHE["nc"]


def _pad_x(xb):
    """[CI, D, H, W] f32 -> host-padded [CI, DP, SLOT] bf16."""
    xh = np.zeros((CI, DP, SLOT_ROWS, WP), dtype=ml_dtypes.bfloat16)
    xh[:, 1 : D + 1, 2 : 2 + H, 0:W] = xb
    return np.ascontiguousarray(xh.reshape(CI, DP, SLOT))


def kernel(x, routing_weights, weight, bias):
    x = np.asarray(x, dtype=np.float32)
    routing_weights = np.asarray(routing_weights, dtype=np.float32)
    weight = np.asarray(weight, dtype=np.float32)
    bias = np.asarray(bias, dtype=np.float32)

    s = routing_weights.sum(axis=1)          # [B]
    w_sum = weight.sum(axis=0)               # [CO, CI, K, K, K]
    b_sum = bias.sum(axis=0)                 # [CO]

    # lhsT layout: [p=(kd,ci), (kh,kw), o]
    wt = np.transpose(w_sum, (2, 1, 3, 4, 0)).reshape(96, NSTEP * CO)

    xbf = x.astype(ml_dtypes.bfloat16)
    in_maps = []
    for b in range(B):
        wb = (s[b] * wt).astype(ml_dtypes.bfloat16)
        bb = np.tile(s[b] * b_sum, 4).reshape(128, 1).astype(np.float32)
        in_maps.append(
            {
                "x": _pad_x(xbf[b]),
                "w": np.ascontiguousarray(wb),
                "bias": bb,
            }
        )

    nc = _get_nc()
    _CACHE["last_in_maps"] = in_maps
    res = run_bass_kernel_spmd(nc, in_maps, list(range(NCORES)))
    _CACHE["last_result"] = res
    out = np.stack(
        [
            np.asarray(res.results[b]["out"], dtype=np.float32).reshape(
                CO, D, H, W
            )
            for b in range(B)
        ]
    )
    return out
